# revision 1
# baseline (speedup 1.0000x reference)
"""Trainium2 Bass kernel for nn_MemorizingTransformer (retrieval_knn).

Sharding: 8 cores = 2 batches x 4 head-pairs. Each core computes attention for
its batch and 2 heads plus its slice of the output projection; the host sums
the 4 partial outputs per batch (the "all-reduce after to_out").

Per-core algorithm (n=2048 tokens, dh=64, 2 heads, kret=32 memories):
  - q/k/v projected with weights stationary (f16) -> qT/kT/vT in PSUM;
    a row-wise pass l2-normalizes q (folding in the logit scale) and k, and
    produces f16 qhatT / khatT plus f16 qhat rows and bf16 v rows.
  - local attention computed TRANSPOSED (keys on partitions): one matmul per
    (key-tile, query-chunk) gives simT; the raw rel-pos bias (fp8, causal
    mask baked in as -100) is accumulated into the same PSUM bank by an
    identity-stationary fp8 matmul, so exp(sim + bias - C) needs no separate
    elementwise multiply.  exp -> bf16 weights feed the PV matmuls directly.
  - memory attention (per-query kNN keys/values): elementwise products on the
    vector engine (2x/4x modes); the q.k segmented reduction runs on the
    TENSOR engine as identity-stationary accumulating matmuls (64 slices of
    [128,32] per tile), the value reduction as a bf16 binary tree on the
    otherwise-idle GPSIMD engine.
  - an extra ones-row of memv / ones-column of v gives both softmax partition
    sums for free; the two branches use different exp shifts and are combined
    with a = exp(Cl-Cm) rescale before the output projection.
  - loop order is chunk-outer / head-inner so input and output DMAs stay
    evenly interleaved; stream pools are allocated below the phase-A pools so
    phase-B DMA prefetch overlaps the projections.  DMA traffic is split
    across the three issuing queues (memk/memv/out on SP, bias on ACT,
    x/weights on GPSIMD) so transfers overlap and no single queue is the
    bottleneck.
"""

import numpy as np
import ml_dtypes
from contextlib import ExitStack

import concourse.bass as bass
import concourse.bacc as bacc
import concourse.mybir as mybir
import concourse.tile as tile
from concourse.masks import make_identity

F32 = mybir.dt.float32
BF16 = mybir.dt.bfloat16
F16 = mybir.dt.float16
F8 = mybir.dt.float8e4
AX = mybir.AxisListType
OP = mybir.AluOpType
ACTF = mybir.ActivationFunctionType

P = 128
DIM = 512
DH = 64
KRET = 32
HPC = 2            # heads per core
NCORES = 8
MASK_NEG = -100.0  # additive causal mask in the fp8 bias plane
C_LOC = 20.75      # >= scale * max|cos| + max|bias|: local exp args <= 0
C_MEM = 60.0       # fixed mem-branch shift; mem logits ~N(0,20) stay
                   # well under exp overflow after -C_MEM
B_LOC = float(np.exp(C_LOC - C_MEM))   # rescale for local sums in combine


def bcast_mid(ap_2d, count):
    """[P, d] AP -> [P, count, d] AP broadcasting a new middle dim (step 0)."""
    return bass.AP(tensor=ap_2d.tensor, offset=ap_2d.offset,
                   ap=[list(ap_2d.ap[0]), [0, count], list(ap_2d.ap[1])])


def pool_tree(nc, pool, prod, rows, width, out_f32, tag):
    """Sum prod[P, rows, width] over the last axis into out_f32 [P, rows]
    via bf16 binary-tree adds on the GPSIMD engine (keeps DVE/ACT free)."""
    cur = prod
    w = width
    with nc.allow_low_precision(reason="bf16 tree partial sums"):
        while w > 2:
            nxt = pool.tile([P, rows, w // 2], cur.dtype, tag=f"{tag}{w}",
                            name=f"{tag}{w}")
            nc.gpsimd.tensor_tensor(out=nxt, in0=cur[:, :, 0:w // 2],
                                    in1=cur[:, :, w // 2:w], op=OP.add)
            cur = nxt
            w //= 2
    nc.gpsimd.tensor_tensor(out=out_f32, in0=cur[:, :, 0],
                            in1=cur[:, :, 1], op=OP.add)


def build_nc(n=2048):
    """Build the per-core Bass program (same NEFF for all 8 cores)."""
    nt = n // P               # 128-token tiles
    nq = n // 512             # 512-query chunks
    nc = bacc.Bacc("TRN2", target_bir_lowering=False, debug=False)

    xt_d = nc.dram_tensor("xt", (DIM, n), F16, kind="ExternalInput").ap()
    wq_d = nc.dram_tensor("wq", (DIM, HPC * DH), F16, kind="ExternalInput").ap()
    wkv_d = nc.dram_tensor("wkv", (DIM, 2 * DH), F16, kind="ExternalInput").ap()
    wout_d = nc.dram_tensor("wout", (HPC * DH, DIM), F16, kind="ExternalInput").ap()
    scales_d = nc.dram_tensor("scales", (1, HPC), F32, kind="ExternalInput").ap()
    memk_d = nc.dram_tensor("memk", (HPC, n, KRET, DH), F16, kind="ExternalInput").ap()
    memv_d = nc.dram_tensor("memv", (HPC, n, DH + 1, KRET), F16, kind="ExternalInput").ap()
    # biasraw[h, c, j, i'] = bias[h, 512c+i', j] (+ MASK_NEG if j > 512c+i')
    bias_d = nc.dram_tensor("biasraw", (HPC, nq, n, 512), F8, kind="ExternalInput").ap()
    out_d = nc.dram_tensor("out", (n, DIM), F16, kind="ExternalOutput").ap()

    with tile.TileContext(nc) as tc, ExitStack() as ctx:
        persist = ctx.enter_context(tc.tile_pool(name="persist", bufs=1))

        # ---- constants -------------------------------------------------
        id_f = persist.tile([P, P], F32)
        make_identity(nc, id_f)
        id_h = persist.tile([P, P], F16)
        make_identity(nc, id_h)
        id_8 = persist.tile([P, P], F8)
        nc.vector.tensor_copy(id_8, id_h)
        scales_sb = persist.tile([P, HPC], F32)
        nc.scalar.dma_start(out=scales_sb, in_=bass.AP(
            tensor=scales_d.tensor, offset=scales_d.offset,
            ap=[[0, P], list(scales_d.ap[1])]))
        wout_sb = persist.tile([P, DIM], F16)
        nc.scalar.dma_start(out=wout_sb, in_=wout_d)
        negc_sb = persist.tile([P, 1], F32)
        nc.vector.memset(negc_sb, -C_LOC)
        negm_sb = persist.tile([P, 1], F32)
        nc.vector.memset(negm_sb, -C_MEM)

        # ---- persistent activations (per-tile for fine-grained deps) --
        qhT_c = [persist.tile([P, 512], F16, name=f"qhT{i}") for i in range(nq)]
        kh2T_t = [persist.tile([P, P], F16, name=f"kh2T{i}") for i in range(nt)]
        qrow_t = [persist.tile([P, P], F16, name=f"qrow{i}") for i in range(nt)]
        vb_t = [persist.tile([P, DH + 1], BF16, name=f"vb{i}") for i in range(nt)]
        a_t = [persist.tile([P, P], F16, name=f"a{i}") for i in range(nt)]

        # phase-B stream pools opened BEFORE phase A so their addresses do
        # not alias phase-A tiles -> DMA prefetch overlaps the projections
        sb2 = ctx.enter_context(tc.tile_pool(name="sb2", bufs=4))
        sb2v = ctx.enter_context(tc.tile_pool(name="sb2v", bufs=4))
        sb3 = ctx.enter_context(tc.tile_pool(name="sb3", bufs=3))
        sbb = ctx.enter_context(tc.tile_pool(name="sbb", bufs=6))
        sbo = ctx.enter_context(tc.tile_pool(name="sbo", bufs=2))
        sc = ctx.enter_context(tc.tile_pool(name="sc", bufs=2))
        sc4 = ctx.enter_context(tc.tile_pool(name="sc4", bufs=4))
        scm = ctx.enter_context(tc.tile_pool(name="scm", bufs=8))
        sct = ctx.enter_context(tc.tile_pool(name="sct", bufs=4))

        # ================= Phase A: projections ========================
        with ExitStack() as actx:
            pa = actx.enter_context(tc.tile_pool(name="pa", bufs=1))
            wq_sb = pa.tile([P, DIM // P, HPC * DH], F16)
            nc.sync.dma_start(out=wq_sb, in_=wq_d.rearrange("(c p) m -> p c m", p=P))
            xt_sb = pa.tile([P, DIM // P, n], F16)
            xt_r = xt_d.rearrange("(c p) n -> p c n", p=P)
            nc.gpsimd.dma_start(out=xt_sb[:, 0, :], in_=xt_r[:, 0, :])
            wkv_sb = pa.tile([P, DIM // P, 2 * DH], F16)
            nc.gpsimd.dma_start(out=wkv_sb, in_=wkv_d.rearrange("(c p) m -> p c m", p=P))
            for cc in range(1, DIM // P):
                nc.gpsimd.dma_start(out=xt_sb[:, cc, :], in_=xt_r[:, cc, :])

            qt_sb = pa.tile([P, n], F16)           # raw qT
            kvt_sb = pa.tile([P, n], F16)          # raw kT|vT
            sq_sb = pa.tile([P, n], F16)           # qT^2 then kT^2 reuse
            sqk_sb = pa.tile([P, n], F16)
            # ones columns selecting [q_h0 dims | q_h1 dims | k dims]
            ones3 = pa.tile([P, 3], F16)
            nc.vector.memset(ones3, 0.0)
            nc.vector.memset(ones3[0:DH, 0:1], 1.0)
            nc.vector.memset(ones3[DH:P, 1:2], 1.0)
            nc.vector.memset(ones3[0:DH, 2:3], 1.0)
            nrmq_sb = pa.tile([2, n], F16)         # |q_h0|^2, |q_h1|^2
            nrmk_sb = pa.tile([1, n], F16)         # |k|^2

            with ExitStack() as pctx:
                psA = pctx.enter_context(tc.tile_pool(name="psA", bufs=1, space="PSUM"))
                q_ps = [psA.tile([P, 512], F32, tag=f"q{t}", name=f"q_ps{t}")
                        for t in range(nq)]
                kv_ps = [psA.tile([P, 512], F32, tag=f"kv{t}", name=f"kv_ps{t}")
                         for t in range(nq)]
                for t in [2, 3, 0, 1]:
                    for c in range(DIM // P):
                        last = c == DIM // P - 1
                        nc.tensor.matmul(q_ps[t], lhsT=wq_sb[:, c, :],
                                         rhs=xt_sb[:, c, bass.ts(t, 512)],
                                         start=(c == 0), stop=last)
                    for c in range(DIM // P):
                        last = c == DIM // P - 1
                        nc.tensor.matmul(kv_ps[t], lhsT=wkv_sb[:, c, :],
                                         rhs=xt_sb[:, c, bass.ts(t, 512)],
                                         start=(c == 0), stop=last)
                with nc.allow_low_precision(reason="f16 projections"):
                    for t in [2, 3, 0, 1]:
                        nc.scalar.copy(qt_sb[:, bass.ts(t, 512)], q_ps[t])
                        nc.vector.tensor_copy(kvt_sb[:, bass.ts(t, 512)], kv_ps[t])
                    # squared rows -> per-query norms via ones-matmuls
                    for t in [2, 3, 0, 1]:
                        nc.scalar.activation(out=sq_sb[:, bass.ts(t, 512)],
                                             in_=q_ps[t], func=ACTF.Square)
                        nc.gpsimd.tensor_tensor(
                            out=sqk_sb[0:DH, bass.ts(t, 512)],
                            in0=kvt_sb[0:DH, bass.ts(t, 512)],
                            in1=kvt_sb[0:DH, bass.ts(t, 512)], op=OP.mult)
                for t in [2, 3, 0, 1]:
                    np_t = psA.tile([2, 512], F32, tag=f"q{t}", name=f"nrm_ps{t}")
                    nk_t = psA.tile([1, 512], F32, tag=f"kv{t}", name=f"nrmk_ps{t}")
                    nc.tensor.matmul(np_t, lhsT=ones3[:, 0:2],
                                     rhs=sq_sb[:, bass.ts(t, 512)],
                                     start=True, stop=True)
                    nc.tensor.matmul(nk_t, lhsT=ones3[0:DH, 2:3],
                                     rhs=sqk_sb[0:DH, bass.ts(t, 512)],
                                     start=True, stop=True)
                    with nc.allow_low_precision(reason="f16 norms"):
                        nc.vector.tensor_copy(nrmq_sb[:, bass.ts(t, 512)], np_t)
                        nc.vector.tensor_copy(nrmk_sb[:, bass.ts(t, 512)], nk_t)

            # row-wise pass: normalize q (x scale) and k, build row tiles
            with ExitStack() as pctx:
                psR = pctx.enter_context(tc.tile_pool(name="psR", bufs=2, space="PSUM"))
                rsb = pctx.enter_context(tc.tile_pool(name="rsb", bufs=3))
                rsc = pctx.enter_context(tc.tile_pool(name="rsc", bufs=4))
                for t in [8, 9, 10, 11, 12, 13, 14, 15, 0, 1, 2, 3, 4, 5, 6, 7]:
                    qr_ps = psR.tile([P, P], F16, tag="qr")
                    nc.tensor.transpose(qr_ps, qt_sb[:, bass.ts(t, P)], id_h)
                    qr_sb = rsb.tile([P, P], F16, tag="qr_sb")
                    nc.scalar.copy(qr_sb, qr_ps)
                    kvr_ps = psR.tile([P, P], F16, tag="kvr")
                    nc.tensor.transpose(kvr_ps, kvt_sb[:, bass.ts(t, P)], id_h)
                    kvr_sb = rsb.tile([P, P], F16, tag="kvr_sb")
                    nc.scalar.copy(kvr_sb, kvr_ps)
                    with nc.allow_low_precision(reason="bf16 values"):
                        nc.gpsimd.tensor_copy(vb_t[t][:, 0:DH], kvr_sb[:, DH:P])

                    nr_ps = psR.tile([P, 4], F16, tag="nr")
                    nc.tensor.transpose(nr_ps[:, 0:2], nrmq_sb[:, bass.ts(t, P)],
                                        id_h[0:2, 0:2])
                    nc.tensor.transpose(nr_ps[:, 2:3], nrmk_sb[:, bass.ts(t, P)],
                                        id_h[0:1, 0:1])
                    nall = rsc.tile([P, 4], F32, tag="nall")
                    nc.scalar.sqrt(nall[:, 0:3], nr_ps[:, 0:3])
                    nc.vector.reciprocal(nall[:, 0:3], nall[:, 0:3])
                    for h in range(HPC):
                        rq = rsc.tile([P, 1], F32, tag="rq")
                        nc.vector.tensor_tensor(out=rq, in0=nall[:, h:h + 1],
                                                in1=scales_sb[:, h:h + 1], op=OP.mult)
                        nc.vector.tensor_scalar_mul(qrow_t[t][:, bass.ts(h, DH)],
                                                    qr_sb[:, bass.ts(h, DH)], rq)
                    qkht_ps = psR.tile([P, 2, P], F16, tag="qkht")
                    nc.tensor.transpose(qkht_ps[:, 0, :], qrow_t[t], id_h)
                    nc.vector.tensor_copy(qhT_c[t // 4][:, bass.ts(t % 4, P)],
                                          qkht_ps[:, 0, :])

                    khr = rsb.tile([P, DH], F16, tag="khr")
                    nc.vector.tensor_scalar_mul(khr, kvr_sb[:, 0:DH], nall[:, 2:3])
                    nc.tensor.transpose(qkht_ps[0:DH, 1, :], khr, id_h)
                    nc.vector.tensor_copy(kh2T_t[t][0:DH, :], qkht_ps[0:DH, 1, :])
                    nc.vector.tensor_copy(kh2T_t[t][DH:P, :], qkht_ps[0:DH, 1, :])
                    nc.vector.memset(vb_t[t][:, DH:DH + 1], 1.0)

        # ================= Phase B: attention ==========================
        with ExitStack() as bctx:
            sim_pool = bctx.enter_context(tc.tile_pool(name="simp", bufs=4, space="PSUM"))
            psm_pool = bctx.enter_context(tc.tile_pool(name="psm", bufs=2, space="PSUM"))

            for c in [2, 1, 3, 0]:
                nkt = 4 * c + 4
                for h in range(HPC):
                    # ---- chunk streams (2-tile pieces for pipelining) -
                    npc = 4 if c == 0 else 2          # pieces per chunk
                    tw = 4 // npc                     # tiles per piece
                    memk_p = []
                    memv_p = []
                    for half in range(npc):
                        mk = sb2.tile([P, tw, KRET, DH], F16, tag="memk",
                                      name=f"memk{half}")
                        kq = (nc.scalar if (c == 2 and h == 0 and half == 0)
                              else nc.sync)
                        kq.dma_start(
                            out=mk,
                            in_=memk_d[h, 512 * c + 128 * tw * half:
                                       512 * c + 128 * tw * (half + 1)].rearrange(
                                "(t p) j d -> p t j d", p=P))
                        memk_p.append(mk)
                        mv = sb2v.tile([P, tw, DH + 1, KRET], F16, tag="memv",
                                       name=f"memv{half}")
                        nc.sync.dma_start(
                            out=mv,
                            in_=memv_d[h, 512 * c + 128 * tw * half:
                                       512 * c + 128 * tw * (half + 1)].rearrange(
                                "(t p) e j -> p t e j", p=P))
                        memv_p.append(mv)
                    bias_p = []
                    for bp in range(nkt // 4):
                        bt = sbb.tile([P, 4, 512], F8, tag="bias",
                                      name=f"bias{bp}")
                        nc.scalar.dma_start(
                            out=bt,
                            in_=bias_d[h, c, 4 * P * bp: 4 * P * (bp + 1),
                                       :].rearrange("(t p) q -> p t q", p=P))
                        bias_p.append(bt)

                    # ---- memory branch: q.k + exp --------------------
                    smem_ps = psm_pool.tile([P, 4, KRET], F32, tag="smem")
                    for g in range(4):
                        it = 4 * c + g
                        kprod = sc.tile([P, KRET, DH], F16, tag="kprod")
                        with nc.allow_low_precision(reason="f16 logit products"):
                            nc.vector.tensor_tensor(
                                out=kprod,
                                in0=bcast_mid(qrow_t[it][:, bass.ts(h, DH)], KRET),
                                in1=memk_p[g // tw][:, g % tw, :, :], op=OP.mult)
                        for d in range(DH):
                            nc.tensor.matmul(smem_ps[:, g, :], lhsT=id_h,
                                             rhs=kprod[:, :, d],
                                             start=(d == 0), stop=(d == DH - 1))
                    wm_sb = sc4.tile([P, 4, KRET], BF16, tag="wm")
                    with nc.allow_low_precision(reason="bf16 softmax weights"):
                        nc.scalar.activation(out=wm_sb, in_=smem_ps,
                                             func=ACTF.Exp, bias=negm_sb)

                    # ---- memory branch: weighted values (GPSIMD) ------
                    mem_res = []
                    for g in range(4):
                        vprod = sc.tile([P, DH + 1, KRET], BF16, tag="vprod")
                        veng = nc.gpsimd if g == 0 else nc.vector
                        with nc.allow_low_precision(reason="bf16 value products"):
                            veng.tensor_tensor(
                                out=vprod, in0=bcast_mid(wm_sb[:, g, :], DH + 1),
                                in1=memv_p[g // tw][:, g % tw, :, :], op=OP.mult)
                        mo_sb = scm.tile([P, DH + 1], F32, tag="mo",
                                         name="mo_sb")
                        pool_tree(nc, sct, vprod, DH + 1, KRET, mo_sb, "vt")
                        mem_res.append(mo_sb)

                    # ---- local branch, transposed --------------------
                    acc_ps = psm_pool.tile([P, 4, DH + 1], F32, tag="acc")
                    for kt in range(nkt):
                        # first valid query column for this key tile (causal)
                        lo = max(0, kt - 4 * c) * P
                        sim_ps = sim_pool.tile([P, 512], F32, tag="sim")
                        nc.tensor.matmul(sim_ps[:, lo:],
                                         lhsT=kh2T_t[kt][bass.ts(h, DH), :],
                                         rhs=qhT_c[c][bass.ts(h, DH), lo:],
                                         start=True, stop=False)
                        nc.tensor.matmul(sim_ps[:, lo:], lhsT=id_8,
                                         rhs=bias_p[kt // 4][:, kt % 4, lo:],
                                         start=False, stop=True)
                        e_sb = sb3.tile([P, 512], BF16, tag="e")
                        nc.scalar.activation(out=e_sb[:, lo:],
                                             in_=sim_ps[:, lo:], func=ACTF.Exp,
                                             bias=negc_sb)
                        for g in range(max(0, kt - 4 * c), 4):
                            it = 4 * c + g
                            nc.tensor.matmul(acc_ps[:, g, :],
                                             lhsT=e_sb[:, bass.ts(g, P)],
                                             rhs=vb_t[kt],
                                             start=(kt == 0), stop=(kt == it))

                    # ---- combine local + memory ----------------------
                    o_sb = None
                    if h == HPC - 1:
                        o_sb = [sbo.tile([P, 2, DIM], F16, tag=f"osb{i}",
                                         name=f"o_sb{i}") for i in range(2)]
                    for g in range(4):
                        it = 4 * c + g
                        slb = scm.tile([P, DH + 1], F32, tag="slb")
                        nc.vector.scalar_tensor_tensor(
                            out=slb, in0=acc_ps[:, g, :], scalar=B_LOC,
                            in1=mem_res[g], op0=OP.mult, op1=OP.add)
                        rz = scm.tile([P, 1], F32, tag="rz")
                        nc.vector.reciprocal(rz, slb[:, DH:DH + 1])
                        nc.vector.tensor_scalar_mul(a_t[it][:, bass.ts(h, DH)],
                                                    slb[:, 0:DH], rz)
                        if h == HPC - 1:
                            # ---- output projection, interleaved ------
                            at_ps = sim_pool.tile([P, P], F16, tag="sim",
                                                  name="at_ps")
                            nc.tensor.transpose(at_ps, a_t[it], id_h)
                            at_sb = sc4.tile([P, P], F16, tag="at_sb")
                            nc.scalar.copy(at_sb, at_ps)
                            o_ps = sim_pool.tile([P, DIM], F32, tag="sim",
                                                 name="o_ps")
                            nc.tensor.matmul(o_ps, lhsT=at_sb, rhs=wout_sb,
                                             start=True, stop=True)
                            nc.scalar.copy(o_sb[g // 2][:, g % 2, :], o_ps)
                            if g % 2 == 1:
                                oq = nc.sync if c in (0, 2) else nc.gpsimd
                                oq.dma_start(
                                    out=out_d[512 * c + 256 * (g // 2):
                                              512 * c + 256 * (g // 2) + 256,
                                              :].rearrange(
                                        "(t p) q -> p t q", p=P),
                                    in_=o_sb[g // 2])

    nc.compile()
    return nc


# ===================== host side =====================================

def prep_core_inputs(x, mem_kv, mem_mask, rel_pos_bias, Wq, Wkv, Wout,
                     scale_param):
    """Shard the full inputs into 8 per-core input maps."""
    b, n, dim = x.shape
    h = scale_param.shape[0]
    nq = n // 512
    f8 = ml_dtypes.float8_e4m3fn

    scales = np.exp(np.asarray(scale_param, np.float32).reshape(h))
    xt = [np.ascontiguousarray(np.asarray(x[i], np.float32).T).astype(np.float16)
          for i in range(b)]
    # raw bias, transposed/blocked: biasT[h, c, j, i'] = bias[h, 512c+i', j],
    # with the causal mask baked in additively (j > 512c+i' -> MASK_NEG)
    braw = np.array(np.asarray(rel_pos_bias[0], np.float32))
    iu = np.triu_indices(n, 1)
    braw[:, iu[0], iu[1]] = MASK_NEG
    biasT = np.ascontiguousarray(
        braw.reshape(h, nq, 512, n).transpose(0, 1, 3, 2)).astype(f8)
    memk = np.asarray(mem_kv[..., 0, :], np.float32).astype(np.float16)
    memv_r = np.asarray(mem_kv[..., 1, :], np.float32).transpose(0, 1, 2, 4, 3)
    memv = np.empty(memv_r.shape[:3] + (memv_r.shape[3] + 1, memv_r.shape[4]),
                    np.float16)
    memv[..., :-1, :] = memv_r.astype(np.float16)
    memv[..., -1, :] = 1.0
    Wq16 = np.asarray(Wq, np.float32).astype(np.float16)
    Wkv16 = np.asarray(Wkv, np.float32).astype(np.float16)
    Wout16 = np.asarray(Wout, np.float32).astype(np.float16)

    in_maps = []
    for c in range(NCORES):
        bi, hg = divmod(c, NCORES // b)
        hs = slice(HPC * hg, HPC * hg + HPC)
        in_maps.append({
            "xt": xt[bi],
            "wq": np.ascontiguousarray(Wq16[:, HPC * DH * hg: HPC * DH * (hg + 1)]),
            "wkv": Wkv16,
            "wout": np.ascontiguousarray(Wout16[HPC * DH * hg: HPC * DH * (hg + 1), :]),
            "scales": np.ascontiguousarray(scales[hs]).reshape(1, HPC),
            "memk": np.ascontiguousarray(memk[bi, hs]),
            "memv": np.ascontiguousarray(memv[bi, hs]),
            "biasraw": np.ascontiguousarray(biasT[hs]),
        })
    return in_maps


_NC_CACHE = {}


def kernel(x, mem_kv, mem_mask, rel_pos_bias, Wq, Wkv, Wout, scale_param,
           trace=False):
    from concourse.bass_utils import run_bass_kernel_spmd

    b, n, dim = x.shape
    in_maps = prep_core_inputs(x, mem_kv, mem_mask, rel_pos_bias, Wq, Wkv,
                               Wout, scale_param)
    if n not in _NC_CACHE:
        _NC_CACHE[n] = build_nc(n)
    nc = _NC_CACHE[n]
    res = run_bass_kernel_spmd(nc, in_maps, core_ids=list(range(NCORES)),
                               trace=trace)
    outs = [r["out"] for r in res.results]
    full = np.zeros((b, n, dim), np.float32)
    g = NCORES // b
    for c in range(NCORES):
        full[c // g] += outs[c].astype(np.float32)
    if trace:
        kernel.last_results = res
    return full



# revision 57
# speedup vs baseline: 1.4630x; 1.4630x over previous
"""Trainium2 Bass kernel for nn_MemorizingTransformer (retrieval_knn).

Sharding: 8 cores = 2 batches x 4 head-pairs. Each core computes attention for
its batch and 2 heads plus its slice of the output projection; the host sums
the 4 partial outputs per batch (the "all-reduce after to_out").

Per-core algorithm (n=2048 tokens, dh=64, 2 heads, kret=32 memories):
  - phase A projects q/kv ROW-major (tokens on partitions): per 128-token tile
    out[tok, inner] accumulates over 4 dim-slices; per-token norms come from a
    DVE square+reduce along the free axis, rsqrt from ACT sqrt + DVE
    reciprocal; normalized rows are transposed per head into qhT (per-head,
    partitions 0..64) and khT column tiles.
  - the memory branch runs as per-token STATIONARY matmuls on the tensor
    engine: for token i, lhsT = memkT slice ([64, 32] kNN keys, packed two
    tokens per 128 partitions at bases 0/64) and rhs = qhat column ->
    smem[:, i] ([32, 1]); exp on ACT gives wm [32, 512] per chunk, which a
    tiled-identity matmul replicates 4x down the partition axis so a second
    per-token matmul with lhsT = memvT slice ([32, 65] values + ones row,
    four tokens per 128 partitions at bases 0/32/64/96, explicit
    tile_position) produces mem_out^T [65, tok] columns. Each tiny matmul
    costs ~1 ns marginal (free-dim-1 output). Tiny matmuls are emitted
    GROUPED by PE tile position: per-instruction tile-position switching
    faults at runtime on real HW.
  - mem_out^T is copied to SBUF (DVE; gpsimd cannot touch PSUM) and
    transposed back to [tok, 65] on the tensor engine, then combined with
    the local branch on DVE (one PSUM operand per DVE op, an HW rule).
  - local attention is TRANSPOSED (keys on partitions): f16 matmul per
    (key-tile, query-chunk), with the raw rel-pos bias added by an fp8
    DoubleRow identity matmul (host stores bias/2 + causal mask; lhsT/rhs use
    stride-0 plane duplication so the DoubleRow sums the plane twice) at half
    the cycle cost; exp -> bf16 weights feed the PV matmuls. The memory
    branch's value stage is interleaved into the local key-tile loop so its
    exp/copy latencies hide under local matmuls.
  - DMA: memkT/memvT/bias are laid out host-side so every transfer uses all
    128 partitions with >= 512B contiguous elements (the cost model charges
    per-partition bytes); transfers are spread over the three DMA queues
    (SP: weights+memk+bias(c<2)+out, ACT: xt(half)+scales, Pool:
    memv+bias(c>=2)) so per-queue serialized transfer time stays balanced.
"""

import numpy as np
import ml_dtypes
from contextlib import ExitStack

import concourse.bass as bass
import concourse.bacc as bacc
import concourse.mybir as mybir
import concourse.tile as tile
from concourse.masks import make_identity

F32 = mybir.dt.float32
BF16 = mybir.dt.bfloat16
F16 = mybir.dt.float16
F8 = mybir.dt.float8e4
AX = mybir.AxisListType
OP = mybir.AluOpType
ACTF = mybir.ActivationFunctionType
DR = mybir.MatmulPerfMode.DoubleRow

P = 128
DIM = 512
DH = 64
KRET = 32
HPC = 2            # heads per core
NCORES = 8
MASK_NEG = -100.0  # additive causal mask (stored as -50 in the bias/2 plane)
C_LOC = 20.75      # >= scale * max|cos| + max|bias|: local exp args <= 0
C_MEM = 60.0       # fixed mem-branch shift
B_LOC = float(np.exp(C_LOC - C_MEM))   # rescale for local sums in combine


def dup2(ap_2d, w=None):
    """[P, d] AP -> [P, 2, d] AP duplicating the plane (stride-0 middle dim),
    for DoubleRow matmuls that should sum the same plane twice."""
    a0, a1 = list(ap_2d.ap[0]), list(ap_2d.ap[1])
    if w is not None:
        a1 = [a1[0], w]
    return bass.AP(tensor=ap_2d.tensor, offset=ap_2d.offset,
                   ap=[a0, [0, 2], a1])


def bcast(ap_nd, count):
    """append a stride-0 dim of size count to an AP (broadcast innermost)."""
    return bass.AP(tensor=ap_nd.tensor, offset=ap_nd.offset,
                   ap=[list(a) for a in ap_nd.ap] + [[0, count]])


def bcastl(ap_nd, count):
    """replace a trailing size-1 dim with a stride-0 dim of size count."""
    aps = [list(a) for a in ap_nd.ap]
    assert aps[-1][1] == 1
    return bass.AP(tensor=ap_nd.tensor, offset=ap_nd.offset,
                   ap=aps[:-1] + [[0, count]])


def build_nc(n=2048, memv_f8=False, use_dr=True, use_tp96=True,
             no_mem=False, no_local=False, no_qk=False, no_wm4=False,
             no_vtiny=False, no_mt=False, vt_n=512, vt_notp=False,
             vt_mod=4, vt_sorted=True, debug=False):
    """Build the per-core Bass program (same NEFF for all 8 cores)."""
    nt = n // P               # 128-token tiles
    nq = n // 512             # 512-query chunks
    MV_DT = F8 if memv_f8 else BF16
    nc = bacc.Bacc("TRN2", target_bir_lowering=False, debug=False)

    xt_d = nc.dram_tensor("xt", (DIM, n), F16, kind="ExternalInput").ap()
    wq_d = nc.dram_tensor("wq", (DIM, HPC * DH), F16, kind="ExternalInput").ap()
    wkv_d = nc.dram_tensor("wkv", (DIM, 2 * DH), F16, kind="ExternalInput").ap()
    wout_d = nc.dram_tensor("wout", (HPC * DH, DIM), F16, kind="ExternalInput").ap()
    scales_d = nc.dram_tensor("scales", (1, HPC), F32, kind="ExternalInput").ap()
    # memkT[h, 64*(i%2)+d, i//2, j] = mem_k[h, i, j, d]  (2 tokens / 128 parts)
    memkT_d = nc.dram_tensor("memkT", (HPC, P, n // 2, KRET), F16, kind="ExternalInput").ap()
    # memvT[h, 32*(i%4)+j, i//4, e] = mem_v[h, i, j, e]  (4 tokens / 128 parts,
    # e=64 is ones)
    memvT_d = nc.dram_tensor("memvT", (HPC, P, n // 4, DH + 1), MV_DT, kind="ExternalInput").ap()
    # biasraw[h, c, j, i'] = 0.5*bias[h, 512c+i', j] (+ -50 if j > 512c+i')
    bias_d = nc.dram_tensor("biasraw", (HPC, nq, n, 512), F8, kind="ExternalInput").ap()
    out_d = nc.dram_tensor("out", (n, DIM), F16, kind="ExternalOutput").ap()
    if debug:
        dwm_d = nc.dram_tensor("dbg_wm", (HPC, nq, KRET, 512), F32,
                               kind="ExternalOutput").ap()
        dslb_d = nc.dram_tensor("dbg_slb", (HPC, nq, P, 4 * (DH + 1)), F32,
                                kind="ExternalOutput").ap()
        dmt_d = nc.dram_tensor("dbg_mt", (HPC, nq, DH + 1, 512), F32,
                               kind="ExternalOutput").ap()
        dacc_d = nc.dram_tensor("dbg_acc", (HPC, nq, P, 4 * (DH + 1)), F32,
                                kind="ExternalOutput").ap()

    with tile.TileContext(nc) as tc, ExitStack() as ctx:
        persist = ctx.enter_context(tc.tile_pool(name="persist", bufs=1))

        # ---- constants -------------------------------------------------
        id_h = persist.tile([P, P], F16)
        make_identity(nc, id_h)
        id_8 = persist.tile([P, P], F8)
        nc.vector.tensor_copy(id_8, id_h)
        id_b = persist.tile([P, P], BF16)
        nc.vector.tensor_copy(id_b, id_h)
        # rep4[j, r] = 1 iff r % 32 == j: replicates a [32, n] tile 4x down
        # the partition axis via one matmul
        rep4 = persist.tile([KRET, P], BF16)
        for r in range(4):
            nc.vector.tensor_copy(rep4[:, bass.ts(r, KRET)], id_h[0:KRET, 0:KRET])
        scales_sb = persist.tile([P, HPC], F32)
        wout_sb = persist.tile([P, DIM], F16)
        negc_sb = persist.tile([P, 1], F32)
        nc.vector.memset(negc_sb, -C_LOC)
        negm_sb = persist.tile([P, 1], F32)
        nc.vector.memset(negm_sb, -C_MEM)

        # ---- persistent activations -----------------------------------
        # qhT2: qhatT duplicated on both partition halves (so the per-token
        # stationary matmuls can address either half with matching offsets)
        qhT = [[persist.tile([P, 512], F16, name=f"qhT{h}_{c}")
                for c in range(nq)] for h in range(HPC)]
        khT_c = [persist.tile([DH, 4, P], F16, name=f"khT{c}") for c in range(nq)]
        vb_c = [persist.tile([P, 4, DH + 1], BF16, name=f"vb{c}") for c in range(nq)]
        a_c = [persist.tile([P, 4, P], F16, name=f"a{c}") for c in range(nq)]

        # phase-B stream pools opened BEFORE phase A so DMA prefetch overlaps
        sbk = ctx.enter_context(tc.tile_pool(name="sbk", bufs=3))
        sbv = ctx.enter_context(tc.tile_pool(name="sbv", bufs=4))
        sbb = ctx.enter_context(tc.tile_pool(name="sbb", bufs=8))
        sbe = ctx.enter_context(tc.tile_pool(name="sbe", bufs=3))
        sbw = ctx.enter_context(tc.tile_pool(name="sbw", bufs=2))
        sbmt = ctx.enter_context(tc.tile_pool(name="sbmt", bufs=2))
        sbo = ctx.enter_context(tc.tile_pool(name="sbo", bufs=2))
        sba = ctx.enter_context(tc.tile_pool(name="sba", bufs=2))

        # ================= Phase A: projections (row-major) ============
        with ExitStack() as actx:
            pa = actx.enter_context(tc.tile_pool(name="pa", bufs=1))
            wq_sb = pa.tile([P, DIM // P, HPC * DH], F16)
            nc.sync.dma_start(out=wq_sb, in_=wq_d.rearrange("(c p) m -> p c m", p=P))
            wkv_sb = pa.tile([P, DIM // P, 2 * DH], F16)
            nc.sync.dma_start(out=wkv_sb, in_=wkv_d.rearrange("(c p) m -> p c m", p=P))
            xt_sb = pa.tile([P, DIM // P, n], F16)
            xt_r = xt_d.rearrange("(c p) n -> p c n", p=P)
            for cc in range(DIM // P):
                (nc.scalar if cc < 2 else nc.sync).dma_start(
                    out=xt_sb[:, cc, :], in_=xt_r[:, cc, :])
            nc.scalar.dma_start(out=scales_sb, in_=bass.AP(
                tensor=scales_d.tensor, offset=scales_d.offset,
                ap=[[0, P], list(scales_d.ap[1])]))
            nc.sync.dma_start(out=wout_sb, in_=wout_d)

            psA = actx.enter_context(tc.tile_pool(name="psA", bufs=2, space="PSUM"))
            rsb = actx.enter_context(tc.tile_pool(name="rsb", bufs=2))
            for c in range(nq):
                q_ps = psA.tile([P, 4, HPC * DH], F32, tag="q", name="q_ps")
                kv_ps = psA.tile([P, 4, 2 * DH], F32, tag="kv", name="kv_ps")
                for tt in range(4):
                    t = 4 * c + tt
                    for cc in range(DIM // P):
                        nc.tensor.matmul(q_ps[:, tt, :],
                                         lhsT=xt_sb[:, cc, bass.ts(t, P)],
                                         rhs=wq_sb[:, cc, :],
                                         start=(cc == 0), stop=(cc == DIM // P - 1))
                    for cc in range(DIM // P):
                        nc.tensor.matmul(kv_ps[:, tt, :],
                                         lhsT=xt_sb[:, cc, bass.ts(t, P)],
                                         rhs=wkv_sb[:, cc, :],
                                         start=(cc == 0), stop=(cc == DIM // P - 1))
                # per-token norms: squares (ACT; only one PSUM input is
                # allowed per DVE op) + reduce along free axis (DVE)
                sq4 = rsb.tile([P, 4, HPC * DH], F32, tag="sq4")
                nc.scalar.square(sq4, q_ps)
                sk4 = rsb.tile([P, 4, DH], F32, tag="sk4")
                nc.scalar.square(sk4, kv_ps[:, :, 0:DH])
                nrm = rsb.tile([P, 4, 3], F32, tag="nrm")
                nc.vector.tensor_reduce(out=nrm[:, :, 0:2],
                                        in_=sq4.rearrange("p t (h d) -> p t h d", h=2),
                                        axis=AX.X, op=OP.add)
                nc.vector.tensor_reduce(out=nrm[:, :, 2:3],
                                        in_=sk4.rearrange("p t (o d) -> p t o d", o=1),
                                        axis=AX.X, op=OP.add)
                rs = rsb.tile([P, 4, 3], F32, tag="rs")
                nc.scalar.sqrt(rs, nrm)
                rr = rsb.tile([P, 4, 3], F32, tag="rr")
                nc.vector.reciprocal(rr, rs)
                rq = rsb.tile([P, 4, HPC], F32, tag="rq")
                nc.vector.tensor_tensor(
                    out=rq, in0=rr[:, :, 0:2],
                    in1=bass.AP(tensor=scales_sb.tensor, offset=scales_sb.offset,
                                ap=[list(scales_sb.ap[0]), [0, 4],
                                    list(scales_sb.ap[1])]),
                    op=OP.mult)
                qrow4 = rsb.tile([P, 4, HPC * DH], F16, tag="qrow4")
                with nc.allow_low_precision(reason="f16 qhat rows"):
                    nc.vector.tensor_tensor(
                        out=qrow4.rearrange("p t (h d) -> p t h d", h=2),
                        in0=q_ps.rearrange("p t (h d) -> p t h d", h=2),
                        in1=bcast(rq, DH), op=OP.mult)
                krow4 = rsb.tile([P, 4, DH], F16, tag="krow4")
                with nc.allow_low_precision(reason="f16 khat rows"):
                    nc.vector.tensor_tensor(
                        out=krow4,
                        in0=kv_ps[:, :, 0:DH],
                        in1=bcastl(rr[:, :, 2:3], DH), op=OP.mult)
                with nc.allow_low_precision(reason="bf16 values"):
                    nc.vector.tensor_copy(vb_c[c][:, :, 0:DH], kv_ps[:, :, DH:2 * DH])
                nc.gpsimd.memset(vb_c[c][:, :, DH:DH + 1], 1.0)

                tq_ps = psA.tile([P, HPC, 4, P], F16, tag="tq", name="tq_ps")
                tk_ps = psA.tile([DH, 4, P], F16, tag="tk", name="tk_ps")
                for tt in range(4):
                    for h in range(HPC):
                        # write BOTH partition halves so the per-token matmuls
                        # can address either half with matching base offsets
                        nc.tensor.transpose(tq_ps[0:DH, h, tt, :],
                                            qrow4[:, tt, bass.ts(h, DH)], id_h)
                        nc.tensor.transpose(tq_ps[DH:P, h, tt, :],
                                            qrow4[:, tt, bass.ts(h, DH)], id_h)
                    nc.tensor.transpose(tk_ps[:, tt, :], krow4[:, tt, :], id_h)
                nc.vector.tensor_copy(
                    qhT[0][c].rearrange("d (t p) -> d t p", t=4),
                    tq_ps[:, 0, :, :])
                nc.scalar.copy(
                    qhT[1][c].rearrange("d (t p) -> d t p", t=4),
                    tq_ps[:, 1, :, :])
                nc.vector.tensor_copy(khT_c[c], tk_ps)

        # ================= Phase B: attention ==========================
        with ExitStack() as bctx:
            simp = bctx.enter_context(tc.tile_pool(name="simp", bufs=2, space="PSUM"))
            psm = bctx.enter_context(tc.tile_pool(name="psm", bufs=1, space="PSUM"))
            psmr = bctx.enter_context(tc.tile_pool(name="psmr", bufs=1, space="PSUM"))

            def issue_mem_dmas(c, h):
                mk = sbk.tile([P, 256, KRET], F16, tag="memk", name=f"mk{c}{h}")
                nc.sync.dma_start(out=mk, in_=memkT_d[h, :, 256 * c:256 * (c + 1), :])
                mv = sbv.tile([P, P, DH + 1], MV_DT, tag="memv", name=f"mv{c}{h}")
                nc.gpsimd.dma_start(out=mv,
                                    in_=memvT_d[h, :, P * c:P * (c + 1), :])
                return mk, mv

            def issue_bias_dmas(c, h):
                bias_p = []
                for bp in range(c + 1):
                    bt = sbb.tile([P, 4, 512], F8, tag="bias", name=f"bias{c}{h}{bp}")
                    (nc.sync if c < 2 else nc.gpsimd).dma_start(
                        out=bt,
                        in_=bias_d[h, c, 4 * P * bp: 4 * P * (bp + 1),
                                   :].rearrange("(t p) q -> p t q", p=P))
                    bias_p.append(bt)
                return bias_p

            # prefetch chunk 0 (both heads)
            pending = {}
            for h in range(HPC):
                pending[(0, h)] = (issue_mem_dmas(0, h), issue_bias_dmas(0, h))

            for c in range(nq):
                nkt = 4 * c + 4
                o_sb = [sbo.tile([P, 2, DIM], F16, tag="osb", name=f"o_sb{c}{i}")
                        for i in range(2)]
                for h in range(HPC):
                    (mk, mv), bias_p = pending.pop((c, h))
                    # prefetch next (c, h)
                    nxt = (c, h + 1) if h + 1 < HPC else (c + 1, 0)
                    if nxt[0] < nq and nxt not in pending:
                        pending[nxt] = (issue_mem_dmas(*nxt), issue_bias_dmas(*nxt))
                    if h == HPC - 1:
                        nxt2 = (c + 1, 1)
                        if nxt2[0] < nq:
                            pending[nxt2] = (issue_mem_dmas(*nxt2),
                                             issue_bias_dmas(*nxt2))

                    # ---- memory branch: per-token q.k matmuls ---------
                    smem_ps = psm.tile([KRET, 512], F32, tag="smem",
                                       name="smem_ps", bufs=2)
                    for i in sorted(range(512), key=lambda i: i % 2):
                        off = DH * (i % 2)
                        nc.tensor.matmul(smem_ps[:, i:i + 1],
                                         lhsT=mk[off:off + DH, i // 2, :],
                                         rhs=qhT[h][c][off:off + DH, i:i + 1],
                                         start=True, stop=True)
                    wm_sb = sbw.tile([KRET, 512], BF16, tag="wm", name="wm_sb")
                    with nc.allow_low_precision(reason="bf16 softmax weights"):
                        nc.scalar.activation(out=wm_sb, in_=smem_ps, func=ACTF.Exp,
                                             bias=negm_sb[0:KRET, :])

                    acc_ps = psm.tile([P, 4, P], F32, tag="acc", name="acc_ps")
                    wm4_sb = sbw.tile([P, 512], BF16, tag="wm4sb", name="wm4_sb")
                    mt_ps = psm.tile([P, 4, DH + 1], BF16, tag="mt", name="mt_ps",
                                     padded_shape=[P, 4, P])

                    def emit_local(kt):
                        lo = max(0, kt - 4 * c) * P
                        sim_ps = simp.tile([P, 512], F32, tag="sim", name="sim_ps")
                        nc.tensor.matmul(sim_ps[:, lo:],
                                         lhsT=khT_c[kt // 4][:, kt % 4, :],
                                         rhs=qhT[h][c][0:DH, lo:],
                                         start=True, stop=False)
                        bb = bias_p[kt // 4][:, kt % 4, lo:]
                        nc.tensor.matmul(sim_ps[:, lo:], lhsT=dup2(id_8),
                                         rhs=dup2(bb), start=False, stop=True,
                                         perf_mode=DR)
                        e_sb = sbe.tile([P, 512], BF16, tag="e", name="e_sb")
                        nc.scalar.activation(out=e_sb[:, lo:], in_=sim_ps[:, lo:],
                                             func=ACTF.Exp, bias=negc_sb)
                        for g in range(max(0, kt - 4 * c), 4):
                            nc.tensor.matmul(acc_ps[:, g, 0:DH + 1],
                                             lhsT=e_sb[:, bass.ts(g, P)],
                                             rhs=vb_c[kt // 4][:, kt % 4, :],
                                             start=(kt == 0), stop=(kt == 4 * c + g),
                                             skip_group_check=True)

                    def emit_wm4():
                        # replicate wm 4x down the partition axis (for the value
                        # matmuls whose stationary tiles sit at offs 0/32/64/96)
                        wm4_ps = psmr.tile([P, 512], F32, tag="wm4", name="wm4_ps")
                        nc.tensor.matmul(wm4_ps, lhsT=rep4, rhs=wm_sb,
                                         start=True, stop=True)
                        with nc.allow_low_precision(reason="bf16 softmax weights"):
                            nc.vector.tensor_copy(wm4_sb, wm4_ps)

                    def emit_vt():
                        # per-token value matmuls, grouped by PE tile position
                        # (per-instruction tile_position switching faults on HW)
                        mr_ps = psmr.tile([DH + 1, 512], F32, tag="mr",
                                          name="mr_ps")
                        for i in sorted(range(512), key=lambda i: i % 4):
                            off = KRET * (i % 4)
                            nc.tensor.matmul(mr_ps[:, i:i + 1],
                                             lhsT=mv[off:off + KRET, i // 4, :],
                                             rhs=wm4_sb[off:off + KRET, i:i + 1],
                                             start=True, stop=True,
                                             tile_position=(off, 0))
                        mt_sb = sbmt.tile([DH + 1, 512], BF16, tag="mt",
                                          name="mt_sb")
                        with nc.allow_low_precision(reason="bf16 mem out"):
                            nc.vector.tensor_copy(mt_sb, mr_ps)
                        for g in range(4):
                            nc.tensor.transpose(mt_ps[:, g, :],
                                                mt_sb[:, bass.ts(g, P)],
                                                id_b[0:DH + 1, 0:DH + 1])

                    # interleave: local tiles hide the exp/copy latencies of
                    # the memory-branch stages
                    for kt in range(nkt):
                        emit_local(kt)
                        if kt == 0:
                            emit_wm4()
                        if kt == 2:
                            emit_vt()
                    if nkt <= 2:
                        emit_vt()

                    # ---- combine local + memory -----------------------
                    if no_local:
                        nc.vector.memset(acc_ps, 1.0)
                    slb0 = sba.tile([P, 4, DH + 1], F32, tag="slb0", name="slb0")
                    nc.vector.tensor_scalar_mul(slb0, acc_ps[:, :, 0:DH + 1],
                                                B_LOC)
                    slb = sba.tile([P, 4, DH + 1], F32, tag="slb", name="slb")
                    nc.vector.tensor_tensor(out=slb, in0=slb0, in1=mt_ps,
                                            op=OP.add)
                    if debug:
                        wmf = sbw.tile([KRET, 512], F32, tag="wmf", name="wmf")
                        nc.vector.tensor_copy(wmf, wm_sb)
                        nc.scalar.dma_start(out=dwm_d[h, c], in_=wmf)
                        nc.scalar.dma_start(
                            out=dslb_d[h, c],
                            in_=slb.rearrange("p a e -> p (a e)"))
                        mtf = sbmt.tile([DH + 1, 512], F32, tag="mtf", name="mtf")
                        nc.vector.tensor_copy(mtf, mt_sb)
                        nc.scalar.dma_start(out=dmt_d[h, c], in_=mtf)
                        accf = sba.tile([P, 4, DH + 1], F32, tag="accf",
                                        name="accf")
                        nc.vector.tensor_copy(accf, acc_ps[:, :, 0:DH + 1])
                        nc.scalar.dma_start(
                            out=dacc_d[h, c],
                            in_=accf.rearrange("p a e -> p (a e)"))
                    rz = sba.tile([P, 4, 1], F32, tag="rz", name="rz")
                    nc.vector.reciprocal(rz, slb[:, :, DH:DH + 1])
                    with nc.allow_low_precision(reason="f16 attention out"):
                        nc.vector.tensor_tensor(
                            out=a_c[c][:, :, DH * h:DH * (h + 1)],
                            in0=slb[:, :, 0:DH],
                            in1=bcastl(rz, DH), op=OP.mult)

                    # ---- output projection ----------------------------
                    if h == HPC - 1:
                        for tt in range(4):
                            at_ps = psm.tile([P, 4, P], F16, tag="mt",
                                             name="at_ps")[:, 0, :]
                            nc.tensor.transpose(at_ps, a_c[c][:, tt, :], id_h)
                            at_sb = sbw.tile([P, P], F16, tag="at_sb", name="at_sb")
                            nc.vector.tensor_copy(at_sb, at_ps)
                            o_ps = psm.tile([P, DIM], F32, tag="smem",
                                            name="o_ps", bufs=2)
                            nc.tensor.matmul(o_ps, lhsT=at_sb, rhs=wout_sb,
                                             start=True, stop=True)
                            with nc.allow_low_precision(reason="f16 output"):
                                nc.vector.tensor_copy(o_sb[tt // 2][:, tt % 2, :],
                                                      o_ps)
                            if tt % 2 == 1:
                                nc.sync.dma_start(
                                    out=out_d[512 * c + 256 * (tt // 2):
                                              512 * c + 256 * (tt // 2) + 256,
                                              :].rearrange("(t p) q -> p t q", p=P),
                                    in_=o_sb[tt // 2])

    nc.compile()
    return nc


# ===================== host side =====================================

def prep_core_inputs(x, mem_kv, mem_mask, rel_pos_bias, Wq, Wkv, Wout,
                     scale_param, memv_f8=False):
    """Shard the full inputs into 8 per-core input maps."""
    b, n, dim = x.shape
    h = scale_param.shape[0]
    nq = n // 512
    f8 = ml_dtypes.float8_e4m3fn
    mv_dt = f8 if memv_f8 else ml_dtypes.bfloat16

    scales = np.exp(np.asarray(scale_param, np.float32).reshape(h))
    xt = [np.ascontiguousarray(np.asarray(x[i], np.float32).T).astype(np.float16)
          for i in range(b)]
    # half-bias, transposed/blocked: biasT[h, c, j, i'] = 0.5*bias[h, 512c+i', j],
    # with the causal mask baked in additively (j > 512c+i' -> MASK_NEG/2)
    braw = np.array(np.asarray(rel_pos_bias[0], np.float32)) * 0.5
    iu = np.triu_indices(n, 1)
    braw[:, iu[0], iu[1]] = MASK_NEG * 0.5
    biasT = np.ascontiguousarray(
        braw.reshape(h, nq, 512, n).transpose(0, 1, 3, 2)).astype(f8)
    kret, dh = mem_kv.shape[3], mem_kv.shape[5]
    # memkT[b, h, 64*(i%2)+d, i//2, j]: 2 tokens per 128 partitions
    memk = np.asarray(mem_kv[..., 0, :], np.float32)          # b h i j d
    memkT = np.ascontiguousarray(
        memk.reshape(b, h, n // 2, 2, kret, dh).transpose(0, 1, 3, 5, 2, 4)
        .reshape(b, h, 2 * dh, n // 2, kret)).astype(np.float16)
    # memvT[b, h, 32*(i%4)+j, i//4, e] with ones at e=64: 4 tokens / 128 parts
    memv = np.asarray(mem_kv[..., 1, :], np.float32)          # b h i j d
    memv_p = np.empty(memv.shape[:4] + (dh + 1,), np.float32)
    memv_p[..., :dh] = memv
    memv_p[..., dh] = 1.0
    memvT = np.ascontiguousarray(
        memv_p.reshape(b, h, n // 4, 4, kret, dh + 1).transpose(0, 1, 3, 4, 2, 5)
        .reshape(b, h, 4 * kret, n // 4, dh + 1)).astype(mv_dt)
    Wq16 = np.asarray(Wq, np.float32).astype(np.float16)
    Wkv16 = np.asarray(Wkv, np.float32).astype(np.float16)
    Wout16 = np.asarray(Wout, np.float32).astype(np.float16)

    in_maps = []
    for c in range(NCORES):
        bi, hg = divmod(c, NCORES // b)
        hs = slice(HPC * hg, HPC * hg + HPC)
        in_maps.append({
            "xt": xt[bi],
            "wq": np.ascontiguousarray(Wq16[:, HPC * DH * hg: HPC * DH * (hg + 1)]),
            "wkv": Wkv16,
            "wout": np.ascontiguousarray(Wout16[HPC * DH * hg: HPC * DH * (hg + 1), :]),
            "scales": np.ascontiguousarray(scales[hs]).reshape(1, HPC),
            "memkT": np.ascontiguousarray(memkT[bi, hs]),
            "memvT": np.ascontiguousarray(memvT[bi, hs]),
            "biasraw": np.ascontiguousarray(biasT[hs]),
        })
    return in_maps


_NC_CACHE = {}
MEMV_F8 = False


def kernel(x, mem_kv, mem_mask, rel_pos_bias, Wq, Wkv, Wout, scale_param,
           trace=False):
    from concourse.bass_utils import run_bass_kernel_spmd

    b, n, dim = x.shape
    in_maps = prep_core_inputs(x, mem_kv, mem_mask, rel_pos_bias, Wq, Wkv,
                               Wout, scale_param, memv_f8=MEMV_F8)
    if n not in _NC_CACHE:
        _NC_CACHE[n] = build_nc(n, memv_f8=MEMV_F8)
    nc = _NC_CACHE[n]
    res = run_bass_kernel_spmd(nc, in_maps, core_ids=list(range(NCORES)),
                               trace=trace)
    outs = [r["out"] for r in res.results]
    full = np.zeros((b, n, dim), np.float32)
    g = NCORES // b
    for c in range(NCORES):
        full[c // g] += outs[c].astype(np.float32)
    if trace:
        kernel.last_results = res
    return full


# revision 60
# speedup vs baseline: 1.5283x; 1.0446x over previous
"""Trainium2 Bass kernel for nn_MemorizingTransformer (retrieval_knn).

Sharding: 8 cores = 2 batches x 4 head-pairs. Each core computes attention for
its batch and 2 heads plus its slice of the output projection; the host sums
the 4 partial outputs per batch (the "all-reduce after to_out").

Per-core algorithm (n=2048 tokens, dh=64, 2 heads, kret=32 memories):
  - phase A projects q/kv ROW-major (tokens on partitions): per 128-token tile
    out[tok, inner] accumulates over 4 dim-slices; per-token norms come from a
    DVE square+reduce along the free axis, rsqrt from ACT sqrt + DVE
    reciprocal; normalized rows are transposed per head into qhT (per-head,
    partitions 0..64) and khT column tiles.
  - the memory branch runs as per-token STATIONARY matmuls on the tensor
    engine: for token i, lhsT = memkT slice ([64, 32] kNN keys, packed two
    tokens per 128 partitions at bases 0/64) and rhs = qhat column ->
    smem[:, i] ([32, 1]); exp on ACT gives wm [32, 512] per chunk, which a
    tiled-identity matmul replicates 4x down the partition axis so a second
    per-token matmul with lhsT = memvT slice ([32, 65] values + ones row,
    four tokens per 128 partitions at bases 0/32/64/96, explicit
    tile_position) produces mem_out^T [65, tok] columns. Each tiny matmul
    costs ~1 ns marginal (free-dim-1 output). Tiny matmuls are emitted
    GROUPED by PE tile position: per-instruction tile-position switching
    faults at runtime on real HW.
  - mem_out^T is copied to SBUF (DVE; gpsimd cannot touch PSUM) and
    transposed back to [tok, 65] on the tensor engine, then combined with
    the local branch on DVE (one PSUM operand per DVE op, an HW rule).
  - local attention is TRANSPOSED (keys on partitions): f16 matmul per
    (key-tile, query-chunk), with the raw rel-pos bias added by an fp8
    DoubleRow identity matmul (host stores bias/2 + causal mask; lhsT/rhs use
    stride-0 plane duplication so the DoubleRow sums the plane twice) at half
    the cycle cost; exp -> bf16 weights feed the PV matmuls. The memory
    branch's value stage is interleaved into the local key-tile loop so its
    exp/copy latencies hide under local matmuls.
  - DMA: memkT/memvT/bias are laid out host-side so every transfer uses all
    128 partitions with >= 512B contiguous elements (the cost model charges
    per-partition bytes); transfers are spread over the three DMA queues
    (SP: weights+memk+bias(c<2)+out, ACT: xt(half)+scales, Pool:
    memv+bias(c>=2)) so per-queue serialized transfer time stays balanced.
"""

import numpy as np
import ml_dtypes
from contextlib import ExitStack

import concourse.bass as bass
import concourse.bacc as bacc
import concourse.mybir as mybir
import concourse.tile as tile
from concourse.masks import make_identity

F32 = mybir.dt.float32
BF16 = mybir.dt.bfloat16
F16 = mybir.dt.float16
F8 = mybir.dt.float8e4
AX = mybir.AxisListType
OP = mybir.AluOpType
ACTF = mybir.ActivationFunctionType
DR = mybir.MatmulPerfMode.DoubleRow

P = 128
DIM = 512
DH = 64
KRET = 32
HPC = 2            # heads per core
NCORES = 8
MASK_NEG = -100.0  # additive causal mask (stored as -50 in the bias/2 plane)
C_LOC = 20.75      # >= scale * max|cos| + max|bias|: local exp args <= 0
C_MEM = 60.0       # fixed mem-branch shift
B_LOC = float(np.exp(C_LOC - C_MEM))   # rescale for local sums in combine


def dup2(ap_2d, w=None):
    """[P, d] AP -> [P, 2, d] AP duplicating the plane (stride-0 middle dim),
    for DoubleRow matmuls that should sum the same plane twice."""
    a0, a1 = list(ap_2d.ap[0]), list(ap_2d.ap[1])
    if w is not None:
        a1 = [a1[0], w]
    return bass.AP(tensor=ap_2d.tensor, offset=ap_2d.offset,
                   ap=[a0, [0, 2], a1])


def bcast(ap_nd, count):
    """append a stride-0 dim of size count to an AP (broadcast innermost)."""
    return bass.AP(tensor=ap_nd.tensor, offset=ap_nd.offset,
                   ap=[list(a) for a in ap_nd.ap] + [[0, count]])


def bcastl(ap_nd, count):
    """replace a trailing size-1 dim with a stride-0 dim of size count."""
    aps = [list(a) for a in ap_nd.ap]
    assert aps[-1][1] == 1
    return bass.AP(tensor=ap_nd.tensor, offset=ap_nd.offset,
                   ap=aps[:-1] + [[0, count]])


def build_nc(n=2048, memv_f8=False, use_dr=True, use_tp96=True,
             no_mem=False, no_local=False, no_qk=False, no_wm4=False,
             no_vtiny=False, no_mt=False, vt_n=512, vt_notp=False,
             vt_mod=4, vt_sorted=True, debug=False):
    """Build the per-core Bass program (same NEFF for all 8 cores)."""
    nt = n // P               # 128-token tiles
    nq = n // 512             # 512-query chunks
    MV_DT = F8 if memv_f8 else BF16
    nc = bacc.Bacc("TRN2", target_bir_lowering=False, debug=False)

    xt_d = nc.dram_tensor("xt", (DIM, n), F16, kind="ExternalInput").ap()
    wq_d = nc.dram_tensor("wq", (DIM, HPC * DH), F16, kind="ExternalInput").ap()
    wkv_d = nc.dram_tensor("wkv", (DIM, 2 * DH), F16, kind="ExternalInput").ap()
    wout_d = nc.dram_tensor("wout", (HPC * DH, DIM), F16, kind="ExternalInput").ap()
    scales_d = nc.dram_tensor("scales", (1, HPC), F32, kind="ExternalInput").ap()
    # memkT[h, 64*(i%2)+d, i//2, j] = mem_k[h, i, j, d]  (2 tokens / 128 parts)
    memkT_d = nc.dram_tensor("memkT", (HPC, P, n // 2, KRET), F16, kind="ExternalInput").ap()
    # memvT[h, 32*(i%4)+j, i//4, e] = mem_v[h, i, j, e]  (4 tokens / 128 parts,
    # e=64 is ones)
    memvT_d = nc.dram_tensor("memvT", (HPC, P, n // 4, DH + 1), MV_DT, kind="ExternalInput").ap()
    # biasraw[h, c, j, i'] = 0.5*bias[h, 512c+i', j] (+ -50 if j > 512c+i')
    bias_d = nc.dram_tensor("biasraw", (HPC, nq, n, 512), F8, kind="ExternalInput").ap()
    out_d = nc.dram_tensor("out", (n, DIM), F16, kind="ExternalOutput").ap()
    if debug:
        dwm_d = nc.dram_tensor("dbg_wm", (HPC, nq, KRET, 512), F32,
                               kind="ExternalOutput").ap()
        dslb_d = nc.dram_tensor("dbg_slb", (HPC, nq, P, 4 * (DH + 1)), F32,
                                kind="ExternalOutput").ap()
        dmt_d = nc.dram_tensor("dbg_mt", (HPC, nq, DH + 1, 512), F32,
                               kind="ExternalOutput").ap()
        dacc_d = nc.dram_tensor("dbg_acc", (HPC, nq, P, 4 * (DH + 1)), F32,
                                kind="ExternalOutput").ap()

    with tile.TileContext(nc) as tc, ExitStack() as ctx:
        persist = ctx.enter_context(tc.tile_pool(name="persist", bufs=1))

        # ---- constants -------------------------------------------------
        id_h = persist.tile([P, P], F16)
        make_identity(nc, id_h)
        id_8 = persist.tile([P, P], F8)
        nc.vector.tensor_copy(id_8, id_h)
        id_b = persist.tile([P, P], BF16)
        nc.vector.tensor_copy(id_b, id_h)
        # rep4[j, r] = 1 iff r % 32 == j: replicates a [32, n] tile 4x down
        # the partition axis via one matmul
        rep4 = persist.tile([KRET, P], BF16)
        for r in range(4):
            nc.vector.tensor_copy(rep4[:, bass.ts(r, KRET)], id_h[0:KRET, 0:KRET])
        scales_sb = persist.tile([P, HPC], F32)
        wout_sb = persist.tile([P, DIM], F16)
        negc_sb = persist.tile([P, 1], F32)
        nc.vector.memset(negc_sb, -C_LOC)
        negm_sb = persist.tile([P, 1], F32)
        nc.vector.memset(negm_sb, -C_MEM)

        # ---- persistent activations -----------------------------------
        # qhT2: qhatT duplicated on both partition halves (so the per-token
        # stationary matmuls can address either half with matching offsets)
        qhT = [[persist.tile([P, 512], F16, name=f"qhT{h}_{c}")
                for c in range(nq)] for h in range(HPC)]
        khT_c = [persist.tile([DH, 4, P], F16, name=f"khT{c}") for c in range(nq)]
        vb_c = [persist.tile([P, 4, DH + 1], BF16, name=f"vb{c}") for c in range(nq)]
        a_c = [persist.tile([P, 4, P], F16, name=f"a{c}") for c in range(nq)]

        # phase-B stream pools opened BEFORE phase A so DMA prefetch overlaps
        sbk = ctx.enter_context(tc.tile_pool(name="sbk", bufs=3))
        sbv = ctx.enter_context(tc.tile_pool(name="sbv", bufs=4))
        sbb = ctx.enter_context(tc.tile_pool(name="sbb", bufs=8))
        sbe = ctx.enter_context(tc.tile_pool(name="sbe", bufs=3))
        sbw = ctx.enter_context(tc.tile_pool(name="sbw", bufs=2))
        sbmt = ctx.enter_context(tc.tile_pool(name="sbmt", bufs=2))
        sbo = ctx.enter_context(tc.tile_pool(name="sbo", bufs=2))
        sba = ctx.enter_context(tc.tile_pool(name="sba", bufs=2))

        # ================= Phase A: projections (row-major) ============
        with ExitStack() as actx:
            pa = actx.enter_context(tc.tile_pool(name="pa", bufs=1))
            wq_sb = pa.tile([P, DIM // P, HPC * DH], F16)
            nc.sync.dma_start(out=wq_sb, in_=wq_d.rearrange("(c p) m -> p c m", p=P))
            wkv_sb = pa.tile([P, DIM // P, 2 * DH], F16)
            nc.sync.dma_start(out=wkv_sb, in_=wkv_d.rearrange("(c p) m -> p c m", p=P))
            xt_sb = pa.tile([P, DIM // P, n], F16)
            xt_r = xt_d.rearrange("(c p) n -> p c n", p=P)
            for cc in range(DIM // P):
                (nc.scalar if cc < 2 else nc.sync).dma_start(
                    out=xt_sb[:, cc, :], in_=xt_r[:, cc, :])
            nc.scalar.dma_start(out=scales_sb, in_=bass.AP(
                tensor=scales_d.tensor, offset=scales_d.offset,
                ap=[[0, P], list(scales_d.ap[1])]))
            nc.sync.dma_start(out=wout_sb, in_=wout_d)

            psA = actx.enter_context(tc.tile_pool(name="psA", bufs=2, space="PSUM"))
            rsb = actx.enter_context(tc.tile_pool(name="rsb", bufs=2))
            for c in range(nq):
                q_ps = psA.tile([P, 4, HPC * DH], F32, tag="q", name="q_ps")
                kv_ps = psA.tile([P, 4, 2 * DH], F32, tag="kv", name="kv_ps")
                for tt in range(4):
                    t = 4 * c + tt
                    for cc in range(DIM // P):
                        nc.tensor.matmul(q_ps[:, tt, :],
                                         lhsT=xt_sb[:, cc, bass.ts(t, P)],
                                         rhs=wq_sb[:, cc, :],
                                         start=(cc == 0), stop=(cc == DIM // P - 1))
                    for cc in range(DIM // P):
                        nc.tensor.matmul(kv_ps[:, tt, :],
                                         lhsT=xt_sb[:, cc, bass.ts(t, P)],
                                         rhs=wkv_sb[:, cc, :],
                                         start=(cc == 0), stop=(cc == DIM // P - 1))
                # per-token norms: squares (ACT; only one PSUM input is
                # allowed per DVE op) + reduce along free axis (DVE)
                sq4 = rsb.tile([P, 4, HPC * DH], F32, tag="sq4")
                nc.scalar.square(sq4, q_ps)
                sk4 = rsb.tile([P, 4, DH], F32, tag="sk4")
                nc.scalar.square(sk4, kv_ps[:, :, 0:DH])
                nrm = rsb.tile([P, 4, 3], F32, tag="nrm")
                nc.vector.tensor_reduce(out=nrm[:, :, 0:2],
                                        in_=sq4.rearrange("p t (h d) -> p t h d", h=2),
                                        axis=AX.X, op=OP.add)
                nc.vector.tensor_reduce(out=nrm[:, :, 2:3],
                                        in_=sk4.rearrange("p t (o d) -> p t o d", o=1),
                                        axis=AX.X, op=OP.add)
                rs = rsb.tile([P, 4, 3], F32, tag="rs")
                nc.scalar.sqrt(rs, nrm)
                rr = rsb.tile([P, 4, 3], F32, tag="rr")
                nc.vector.reciprocal(rr, rs)
                rq = rsb.tile([P, 4, HPC], F32, tag="rq")
                nc.vector.tensor_tensor(
                    out=rq, in0=rr[:, :, 0:2],
                    in1=bass.AP(tensor=scales_sb.tensor, offset=scales_sb.offset,
                                ap=[list(scales_sb.ap[0]), [0, 4],
                                    list(scales_sb.ap[1])]),
                    op=OP.mult)
                qrow4 = rsb.tile([P, 4, HPC * DH], F16, tag="qrow4")
                with nc.allow_low_precision(reason="f16 qhat rows"):
                    nc.vector.tensor_tensor(
                        out=qrow4.rearrange("p t (h d) -> p t h d", h=2),
                        in0=q_ps.rearrange("p t (h d) -> p t h d", h=2),
                        in1=bcast(rq, DH), op=OP.mult)
                krow4 = rsb.tile([P, 4, DH], F16, tag="krow4")
                with nc.allow_low_precision(reason="f16 khat rows"):
                    nc.vector.tensor_tensor(
                        out=krow4,
                        in0=kv_ps[:, :, 0:DH],
                        in1=bcastl(rr[:, :, 2:3], DH), op=OP.mult)
                with nc.allow_low_precision(reason="bf16 values"):
                    nc.vector.tensor_copy(vb_c[c][:, :, 0:DH], kv_ps[:, :, DH:2 * DH])
                nc.gpsimd.memset(vb_c[c][:, :, DH:DH + 1], 1.0)

                tq_ps = psA.tile([P, HPC, 4, P], F16, tag="tq", name="tq_ps")
                tk_ps = psA.tile([DH, 4, P], F16, tag="tk", name="tk_ps")
                for tt in range(4):
                    for h in range(HPC):
                        # write BOTH partition halves so the per-token matmuls
                        # can address either half with matching base offsets
                        nc.tensor.transpose(tq_ps[0:DH, h, tt, :],
                                            qrow4[:, tt, bass.ts(h, DH)], id_h)
                        nc.tensor.transpose(tq_ps[DH:P, h, tt, :],
                                            qrow4[:, tt, bass.ts(h, DH)], id_h)
                    nc.tensor.transpose(tk_ps[:, tt, :], krow4[:, tt, :], id_h)
                nc.vector.tensor_copy(
                    qhT[0][c].rearrange("d (t p) -> d t p", t=4),
                    tq_ps[:, 0, :, :])
                nc.scalar.copy(
                    qhT[1][c].rearrange("d (t p) -> d t p", t=4),
                    tq_ps[:, 1, :, :])
                nc.vector.tensor_copy(khT_c[c], tk_ps)

        # ================= Phase B: attention ==========================
        with ExitStack() as bctx:
            simp = bctx.enter_context(tc.tile_pool(name="simp", bufs=2, space="PSUM"))
            psm = bctx.enter_context(tc.tile_pool(name="psm", bufs=1, space="PSUM"))

            def issue_mem_dmas(c, h):
                mk = sbk.tile([P, 256, KRET], F16, tag="memk", name=f"mk{c}{h}")
                nc.sync.dma_start(out=mk, in_=memkT_d[h, :, 256 * c:256 * (c + 1), :])
                mv = sbv.tile([P, P, DH + 1], MV_DT, tag="memv", name=f"mv{c}{h}")
                nc.gpsimd.dma_start(out=mv,
                                    in_=memvT_d[h, :, P * c:P * (c + 1), :])
                return mk, mv

            def issue_bias_dmas(c, h):
                bias_p = []
                for bp in range(c + 1):
                    bt = sbb.tile([P, 4, 512], F8, tag="bias", name=f"bias{c}{h}{bp}")
                    (nc.sync if c < 1 else nc.gpsimd).dma_start(
                        out=bt,
                        in_=bias_d[h, c, 4 * P * bp: 4 * P * (bp + 1),
                                   :].rearrange("(t p) q -> p t q", p=P))
                    bias_p.append(bt)
                return bias_p

            # prefetch chunk 0 (both heads)
            pending = {}
            for h in range(HPC):
                pending[(0, h)] = (issue_mem_dmas(0, h), issue_bias_dmas(0, h))

            for c in range(nq):
                nkt = 4 * c + 4
                o_sb = [sbo.tile([P, 2, DIM], F16, tag="osb", name=f"o_sb{c}{i}")
                        for i in range(2)]
                for h in range(HPC):
                    (mk, mv), bias_p = pending.pop((c, h))
                    # prefetch next (c, h)
                    nxt = (c, h + 1) if h + 1 < HPC else (c + 1, 0)
                    if nxt[0] < nq and nxt not in pending:
                        pending[nxt] = (issue_mem_dmas(*nxt), issue_bias_dmas(*nxt))
                    if h == HPC - 1:
                        nxt2 = (c + 1, 1)
                        if nxt2[0] < nq:
                            pending[nxt2] = (issue_mem_dmas(*nxt2),
                                             issue_bias_dmas(*nxt2))

                    # ---- memory branch: per-token q.k matmuls ---------
                    smem_ps = psm.tile([KRET, 512], F32, tag="smem",
                                       name="smem_ps", bufs=2)
                    for i in sorted(range(512), key=lambda i: i % 2):
                        off = DH * (i % 2)
                        nc.tensor.matmul(smem_ps[:, i:i + 1],
                                         lhsT=mk[off:off + DH, i // 2, :],
                                         rhs=qhT[h][c][off:off + DH, i:i + 1],
                                         start=True, stop=True)
                    wm_sb = sbw.tile([KRET, 512], BF16, tag="wm", name="wm_sb")
                    with nc.allow_low_precision(reason="bf16 softmax weights"):
                        nc.scalar.activation(out=wm_sb, in_=smem_ps, func=ACTF.Exp,
                                             bias=negm_sb[0:KRET, :])

                    acc_ps = psm.tile([P, 4, P], F32, tag="acc", name="acc_ps")
                    wm4_sb = sbw.tile([P, 512], BF16, tag="wm4sb", name="wm4_sb")
                    mt_ps = psm.tile([P, 4, DH + 1], BF16, tag="mt", name="mt_ps",
                                     padded_shape=[P, 4, P])

                    def emit_local(kts):
                        # one or two key tiles share a 2-bank sim tile and a
                        # single exp (amortizes the ACT per-instr overhead)
                        sim_ps = simp.tile([P, 2, 512], F32, tag="sim",
                                           name="sim_ps")
                        e_sb = sbe.tile([P, 2, 512], BF16, tag="e", name="e_sb")
                        for pl, kt in enumerate(kts):
                            lo = max(0, kt - 4 * c) * P
                            nc.tensor.matmul(sim_ps[:, pl, lo:],
                                             lhsT=khT_c[kt // 4][:, kt % 4, :],
                                             rhs=qhT[h][c][0:DH, lo:],
                                             start=True, stop=False)
                            bb = bias_p[kt // 4][:, kt % 4, lo:]
                            nc.tensor.matmul(sim_ps[:, pl, lo:], lhsT=dup2(id_8),
                                             rhs=dup2(bb), start=False, stop=True,
                                             perf_mode=DR)
                        lo0 = max(0, kts[0] - 4 * c) * P
                        if len(kts) == 2:
                            nc.scalar.activation(out=e_sb, in_=sim_ps,
                                                 func=ACTF.Exp, bias=negc_sb)
                        else:
                            nc.scalar.activation(out=e_sb[:, 0, lo0:],
                                                 in_=sim_ps[:, 0, lo0:],
                                                 func=ACTF.Exp, bias=negc_sb)
                        for pl, kt in enumerate(kts):
                            for g in range(max(0, kt - 4 * c), 4):
                                nc.tensor.matmul(acc_ps[:, g, 0:DH + 1],
                                                 lhsT=e_sb[:, pl, bass.ts(g, P)],
                                                 rhs=vb_c[kt // 4][:, kt % 4, :],
                                                 start=(kt == 0),
                                                 stop=(kt == 4 * c + g),
                                                 skip_group_check=True)

                    def emit_wm4():
                        # replicate wm 4x down the partition axis (for the value
                        # matmuls whose stationary tiles sit at offs 0/32/64/96)
                        wm4_ps = psm.tile([P, 512], F32, tag="smem",
                                          name="wm4_ps", bufs=2)
                        nc.tensor.matmul(wm4_ps, lhsT=rep4, rhs=wm_sb,
                                         start=True, stop=True)
                        with nc.allow_low_precision(reason="bf16 softmax weights"):
                            nc.vector.tensor_copy(wm4_sb, wm4_ps)

                    def emit_vt():
                        # per-token value matmuls, grouped by PE tile position
                        # (per-instruction tile_position switching faults on HW)
                        mr_ps = psm.tile([DH + 1, 512], F32, tag="smem",
                                         name="mr_ps", bufs=2)
                        for i in sorted(range(512), key=lambda i: i % 4):
                            off = KRET * (i % 4)
                            nc.tensor.matmul(mr_ps[:, i:i + 1],
                                             lhsT=mv[off:off + KRET, i // 4, :],
                                             rhs=wm4_sb[off:off + KRET, i:i + 1],
                                             start=True, stop=True,
                                             tile_position=(off, 0))
                        mt_sb = sbmt.tile([DH + 1, 512], BF16, tag="mt",
                                          name="mt_sb")
                        with nc.allow_low_precision(reason="bf16 mem out"):
                            nc.vector.tensor_copy(mt_sb, mr_ps)
                        for g in range(4):
                            nc.tensor.transpose(mt_ps[:, g, :],
                                                mt_sb[:, bass.ts(g, P)],
                                                id_b[0:DH + 1, 0:DH + 1])

                    # interleave: local tiles hide the exp/copy latencies of
                    # the memory-branch stages
                    groups = []
                    full = list(range(4 * c))
                    while len(full) >= 2:
                        groups.append((full.pop(0), full.pop(0)))
                    groups += [(kt,) for kt in full]
                    groups += [(kt,) for kt in range(4 * c, nkt)]
                    for gi, kts in enumerate(groups):
                        emit_local(kts)
                        if gi == 0:
                            emit_wm4()
                        if gi == 1:
                            emit_vt()
                    if len(groups) <= 1:
                        emit_vt()

                    # ---- combine local + memory -----------------------
                    if no_local:
                        nc.vector.memset(acc_ps, 1.0)
                    slb0 = sba.tile([P, 4, DH + 1], F32, tag="slb0", name="slb0")
                    nc.vector.tensor_scalar_mul(slb0, acc_ps[:, :, 0:DH + 1],
                                                B_LOC)
                    slb = sba.tile([P, 4, DH + 1], F32, tag="slb", name="slb")
                    nc.vector.tensor_tensor(out=slb, in0=slb0, in1=mt_ps,
                                            op=OP.add)
                    if debug:
                        wmf = sbw.tile([KRET, 512], F32, tag="wmf", name="wmf")
                        nc.vector.tensor_copy(wmf, wm_sb)
                        nc.scalar.dma_start(out=dwm_d[h, c], in_=wmf)
                        nc.scalar.dma_start(
                            out=dslb_d[h, c],
                            in_=slb.rearrange("p a e -> p (a e)"))
                        mtf = sbmt.tile([DH + 1, 512], F32, tag="mtf", name="mtf")
                        nc.vector.tensor_copy(mtf, mt_sb)
                        nc.scalar.dma_start(out=dmt_d[h, c], in_=mtf)
                        accf = sba.tile([P, 4, DH + 1], F32, tag="accf",
                                        name="accf")
                        nc.vector.tensor_copy(accf, acc_ps[:, :, 0:DH + 1])
                        nc.scalar.dma_start(
                            out=dacc_d[h, c],
                            in_=accf.rearrange("p a e -> p (a e)"))
                    rz = sba.tile([P, 4, 1], F32, tag="rz", name="rz")
                    nc.vector.reciprocal(rz, slb[:, :, DH:DH + 1])
                    with nc.allow_low_precision(reason="f16 attention out"):
                        nc.vector.tensor_tensor(
                            out=a_c[c][:, :, DH * h:DH * (h + 1)],
                            in0=slb[:, :, 0:DH],
                            in1=bcastl(rz, DH), op=OP.mult)

                    # ---- output projection ----------------------------
                    if h == HPC - 1:
                        for tt in range(4):
                            at_ps = psm.tile([P, 4, P], F16, tag="mt",
                                             name="at_ps")[:, 0, :]
                            nc.tensor.transpose(at_ps, a_c[c][:, tt, :], id_h)
                            at_sb = sbw.tile([P, P], F16, tag="at_sb", name="at_sb")
                            nc.vector.tensor_copy(at_sb, at_ps)
                            o_ps = psm.tile([P, DIM], F32, tag="smem",
                                            name="o_ps", bufs=2)
                            nc.tensor.matmul(o_ps, lhsT=at_sb, rhs=wout_sb,
                                             start=True, stop=True)
                            with nc.allow_low_precision(reason="f16 output"):
                                nc.vector.tensor_copy(o_sb[tt // 2][:, tt % 2, :],
                                                      o_ps)
                            if tt % 2 == 1:
                                nc.sync.dma_start(
                                    out=out_d[512 * c + 256 * (tt // 2):
                                              512 * c + 256 * (tt // 2) + 256,
                                              :].rearrange("(t p) q -> p t q", p=P),
                                    in_=o_sb[tt // 2])

    nc.compile()
    return nc


# ===================== host side =====================================

def prep_core_inputs(x, mem_kv, mem_mask, rel_pos_bias, Wq, Wkv, Wout,
                     scale_param, memv_f8=False):
    """Shard the full inputs into 8 per-core input maps."""
    b, n, dim = x.shape
    h = scale_param.shape[0]
    nq = n // 512
    f8 = ml_dtypes.float8_e4m3fn
    mv_dt = f8 if memv_f8 else ml_dtypes.bfloat16

    scales = np.exp(np.asarray(scale_param, np.float32).reshape(h))
    xt = [np.ascontiguousarray(np.asarray(x[i], np.float32).T).astype(np.float16)
          for i in range(b)]
    # half-bias, transposed/blocked: biasT[h, c, j, i'] = 0.5*bias[h, 512c+i', j],
    # with the causal mask baked in additively (j > 512c+i' -> MASK_NEG/2)
    braw = np.array(np.asarray(rel_pos_bias[0], np.float32)) * 0.5
    iu = np.triu_indices(n, 1)
    braw[:, iu[0], iu[1]] = MASK_NEG * 0.5
    biasT = np.ascontiguousarray(
        braw.reshape(h, nq, 512, n).transpose(0, 1, 3, 2)).astype(f8)
    kret, dh = mem_kv.shape[3], mem_kv.shape[5]
    # memkT[b, h, 64*(i%2)+d, i//2, j]: 2 tokens per 128 partitions
    memk = np.asarray(mem_kv[..., 0, :], np.float32)          # b h i j d
    memkT = np.ascontiguousarray(
        memk.reshape(b, h, n // 2, 2, kret, dh).transpose(0, 1, 3, 5, 2, 4)
        .reshape(b, h, 2 * dh, n // 2, kret)).astype(np.float16)
    # memvT[b, h, 32*(i%4)+j, i//4, e] with ones at e=64: 4 tokens / 128 parts
    memv = np.asarray(mem_kv[..., 1, :], np.float32)          # b h i j d
    memv_p = np.empty(memv.shape[:4] + (dh + 1,), np.float32)
    memv_p[..., :dh] = memv
    memv_p[..., dh] = 1.0
    memvT = np.ascontiguousarray(
        memv_p.reshape(b, h, n // 4, 4, kret, dh + 1).transpose(0, 1, 3, 4, 2, 5)
        .reshape(b, h, 4 * kret, n // 4, dh + 1)).astype(mv_dt)
    Wq16 = np.asarray(Wq, np.float32).astype(np.float16)
    Wkv16 = np.asarray(Wkv, np.float32).astype(np.float16)
    Wout16 = np.asarray(Wout, np.float32).astype(np.float16)

    in_maps = []
    for c in range(NCORES):
        bi, hg = divmod(c, NCORES // b)
        hs = slice(HPC * hg, HPC * hg + HPC)
        in_maps.append({
            "xt": xt[bi],
            "wq": np.ascontiguousarray(Wq16[:, HPC * DH * hg: HPC * DH * (hg + 1)]),
            "wkv": Wkv16,
            "wout": np.ascontiguousarray(Wout16[HPC * DH * hg: HPC * DH * (hg + 1), :]),
            "scales": np.ascontiguousarray(scales[hs]).reshape(1, HPC),
            "memkT": np.ascontiguousarray(memkT[bi, hs]),
            "memvT": np.ascontiguousarray(memvT[bi, hs]),
            "biasraw": np.ascontiguousarray(biasT[hs]),
        })
    return in_maps


_NC_CACHE = {}
MEMV_F8 = False


def kernel(x, mem_kv, mem_mask, rel_pos_bias, Wq, Wkv, Wout, scale_param,
           trace=False):
    from concourse.bass_utils import run_bass_kernel_spmd

    b, n, dim = x.shape
    in_maps = prep_core_inputs(x, mem_kv, mem_mask, rel_pos_bias, Wq, Wkv,
                               Wout, scale_param, memv_f8=MEMV_F8)
    if n not in _NC_CACHE:
        _NC_CACHE[n] = build_nc(n, memv_f8=MEMV_F8)
    nc = _NC_CACHE[n]
    res = run_bass_kernel_spmd(nc, in_maps, core_ids=list(range(NCORES)),
                               trace=trace)
    outs = [r["out"] for r in res.results]
    full = np.zeros((b, n, dim), np.float32)
    g = NCORES // b
    for c in range(NCORES):
        full[c // g] += outs[c].astype(np.float32)
    if trace:
        kernel.last_results = res
    return full
